# revision 12
# baseline (speedup 1.0000x reference)
"""Multi-head Koopman module on 8 Trainium2 NeuronCores (v3).

Sharding:
  phase 1: core c -> batch b = c//2, head block h0 = 8*(c%2); computes the
    per-(k,h) Gram G, shifted Gram M and cross-covariance C_v over the prefix.
  host:    batched 48x48 cholesky/inverse/svd -> E = gate * B_v L A^2 L^-1,
    folded into a single output operator W_eff[D, H*HD] per batch.
  phase 2: core c -> batch b = c//2, token half t = c%2; computes
    y = normed @ W_eff @ W_O for its 1024 tokens (full head contraction,
    so outputs concatenate with no cross-core reduction).

Data path: bf16 activations/weights (fp32 PSUM accumulation) — halves DMA
traffic, every matmul at 1 cycle/row.  LayerNorm never blocks the PE: raw
x tiles are transposed as soon as they land; the mean folds in as a
rank-1 PSUM correction (-mu x colsum(W)), pushed to the LAST stage of
each chain so no accumulation group waits on LN; 1/std is the per-token
scale of the PSUM->SBUF copy.  Phase 1 interleaves the V and op-0 K
projections into the transpose loop (PE work rate matches the ACT LN-stat
rate); G/M/C_v are per-head 48-wide bf16 matmuls packed into 3 DMAs per
operator.  Phase 2 streams the accT accumulation with the W_eff chunk
arrivals (i-outer, 4 PSUM banks per half-pass) and slots the second token
chunk's transposes between the halves so the PE stream has no stage
boundaries.
"""

import math

import numpy as np

B, T, D = 4, 2048, 1024
H, HD = 16, 64
K_OPS, R = 4, 48
LN_EPS = 1e-5
NCORES = 8
HPC = H // 2          # heads per core in phase 1 = 8
NKQ = HPC * R         # 384
NV = HPC * HD         # 512
ND = D // 128         # 8 d-chunks
TH = T // 2           # tokens per core in phase 2 = 1024
NTH = TH // 128       # 8 token tiles per core in phase 2

GB = [0, 64, 128]       # G/M chunk rhs col bases over the 384 key dims
CVB = [0, 128, 256]     # C_v^T chunk rhs col bases over the 512 value dims

_cache = {}
LAST_PERF = {}


def _bf16(a):
    import ml_dtypes
    return np.asarray(a, np.float32).astype(ml_dtypes.bfloat16)


def _blocks_for_head(h):
    r0, r1 = R * h, R * h + R
    out = []
    for c in range(3):
        lo, hi = max(r0, 128 * c), min(r1, 128 * (c + 1))
        if lo < hi:
            out.append((c, lo - 128 * c, hi - lo, lo - r0))
    return out


def _split_multi_waits(nc):
    """walrus codegen accepts at most one sync wait per instruction; move
    extra waits onto preceding wait-only NoOps on the same engine."""
    from concourse import mybir
    for fn in nc.m.functions:
        for bb in fn.blocks:
            insts = list(bb.instructions)
            new = []
            changed = False
            for inst in insts:
                si = inst.sync_info
                if si is not None and si.on_wait and len(si.on_wait) > 1:
                    waits = list(si.on_wait)
                    for j, w in enumerate(waits[:-1]):
                        new.append(mybir.InstNoOp(
                            name=f"{inst.name}-ws{j}", engine=inst.engine,
                            ins=[], outs=[],
                            sync_info=mybir.SyncInfo(on_wait=[w], on_update=[])))
                    inst.sync_info = mybir.SyncInfo(on_wait=[waits[-1]],
                                                    on_update=list(si.on_update))
                    changed = True
                new.append(inst)
            if changed:
                bb.instructions = new
    return nc


def _emit_ln_stats(nc, mybir, lnp, eps_t, x_tile, it, negmu_row, rstds, inv_d):
    """LN stats only (PE-independent): ACT accum passes -> negmu/rstd.
    negmu lands (bf16) in negmu_row[0, it*128:(it+1)*128] via a transposing
    DMA; rstd stays as a per-tile [128, 1] f32 column (ACT copy scale)."""
    f32 = mybir.dt.float32
    bf16 = mybir.dt.bfloat16
    xsq = lnp.tile([128, 2, 512], f32, tag="xsq", name="xsq")
    ssum = lnp.tile([128, 1], f32, tag="ss", name="ssum")
    ssq = lnp.tile([128, 1], f32, tag="sq", name="ssq")
    nc.scalar.activation(out=xsq, in_=x_tile,
                         func=mybir.ActivationFunctionType.Copy,
                         bias=0.0, scale=1.0, accum_out=ssum)
    nc.scalar.activation(out=xsq, in_=x_tile,
                         func=mybir.ActivationFunctionType.Square,
                         bias=0.0, scale=1.0, accum_out=ssq)
    negmu = lnp.tile([128, 1], f32, tag="nm", name="negmu")
    nc.vector.tensor_scalar_mul(negmu, ssum, -inv_d)
    negmu_b = lnp.tile([128, 1], bf16, tag="nb", name="negmu_b")
    nc.vector.tensor_copy(out=negmu_b, in_=negmu)
    nc.sync.dma_start(out=negmu_row[0:1, it * 128:(it + 1) * 128],
                      in_=negmu_b)
    musq = lnp.tile([128, 1], f32, tag="m2", name="musq")
    nc.vector.tensor_tensor(out=musq, in0=negmu, in1=negmu,
                            op=mybir.AluOpType.mult)
    var = lnp.tile([128, 1], f32, tag="va", name="var")
    nc.vector.scalar_tensor_tensor(
        out=var, in0=ssq, scalar=inv_d, in1=musq,
        op0=mybir.AluOpType.mult, op1=mybir.AluOpType.subtract)
    std = lnp.tile([128, 1], f32, tag="sd", name="std")
    nc.scalar.activation(out=std, in_=var,
                         func=mybir.ActivationFunctionType.Sqrt,
                         bias=eps_t[:, 0:1], scale=1.0)
    rstd = rstds[it]
    nc.vector.reciprocal(rstd, std)


def _build_phase1(pt: int):
    """pt = number of 128-row prefix tiles; input xp is [pt*128, D] bf16
    with rows >= prefix_len zeroed on the host."""
    import concourse.bass as bass
    import concourse.tile as tile
    from concourse import mybir
    from concourse.masks import make_identity
    from contextlib import ExitStack

    f32 = mybir.dt.float32
    bf16 = mybir.dt.bfloat16
    PTT = pt * 128

    nc = bass.Bass()
    xp = nc.dram_tensor("xp", [PTT, D], bf16, kind="ExternalInput")
    wk = nc.dram_tensor("wk", [K_OPS, D, NKQ], bf16, kind="ExternalInput")
    wv = nc.dram_tensor("wv", [D, NV], bf16, kind="ExternalInput")
    csk = nc.dram_tensor("csk", [K_OPS, 1, NKQ], bf16, kind="ExternalInput")
    csv = nc.dram_tensor("csv", [1, NV], bf16, kind="ExternalInput")
    g_out = nc.dram_tensor("g_out", [K_OPS, R, HPC, R], f32,
                           kind="ExternalOutput")
    m_out = nc.dram_tensor("m_out", [K_OPS, R, HPC, R], f32,
                           kind="ExternalOutput")
    cv_out = nc.dram_tensor("cv_out", [K_OPS, R, HPC, HD], f32,
                            kind="ExternalOutput")

    mm = nc.tensor.matmul

    with tile.TileContext(nc) as tc, ExitStack() as ctx:
        const = ctx.enter_context(tc.tile_pool(name="const", bufs=1))
        xtp = ctx.enter_context(tc.tile_pool(name="xtp", bufs=1))
        xin = ctx.enter_context(tc.tile_pool(name="xin", bufs=6))
        lnp = ctx.enter_context(tc.tile_pool(name="lnp", bufs=3))
        rsp = ctx.enter_context(tc.tile_pool(name="rsp", bufs=1))
        valsp = ctx.enter_context(tc.tile_pool(name="valsp", bufs=1))
        keysp = ctx.enter_context(tc.tile_pool(name="keysp", bufs=2))
        gmsb = ctx.enter_context(tc.tile_pool(name="gmsb", bufs=2))
        wvp = ctx.enter_context(tc.tile_pool(name="wvp", bufs=1))
        wkp = ctx.enter_context(tc.tile_pool(name="wkp", bufs=1))

        ident_f = const.tile([128, 128], f32)
        make_identity(nc, ident_f)
        ident = const.tile([128, 128], bf16)
        nc.vector.tensor_copy(out=ident, in_=ident_f)
        eps_t = const.tile([128, 1], f32)
        nc.vector.memset(eps_t, LN_EPS)
        zrow_f = const.tile([1, NKQ], f32)
        nc.vector.memset(zrow_f, 0.0)
        zrow = const.tile([1, NKQ], bf16)
        nc.vector.tensor_copy(out=zrow, in_=zrow_f)
        negmu_row = const.tile([1, PTT], bf16)
        rstds = [rsp.tile([128, 1], f32, tag=f"r{it}", name=f"rstd{it}")
                 for it in range(pt)]

        # raw-x^T tiles (normalization folded into the projections)
        xT = [xtp.tile([128, PTT], bf16, tag=f"xT{d}", name=f"xT{d}")
              for d in range(ND)]

        xp_r = xp.rearrange("(t p) (a b) -> t p a b", p=128, a=2)

        # DMA order: x tiles stream first (gate transposes from t=0), then
        # the projection weights (needed only after the transposes)
        x_tiles = []
        for it in range(min(4, pt)):
            x_tile = xin.tile([128, 2, 512], bf16, tag="x", name=f"xt{it}")
            nc.sync.dma_start(out=x_tile, in_=xp_r[it])
            x_tiles.append(x_tile)
        csv_sb = const.tile([1, NV], bf16)
        nc.sync.dma_start(out=csv_sb, in_=csv[:, :])
        csk_sb = const.tile([1, K_OPS, NKQ], bf16)
        nc.sync.dma_start(out=csk_sb, in_=csk.rearrange("k o n -> o k n"))
        wv_sb = wvp.tile([128, ND, NV], bf16)
        nc.sync.dma_start(out=wv_sb, in_=wv.rearrange("(a p) n -> p a n", p=128))
        for it in range(4, pt):
            x_tile = xin.tile([128, 2, 512], bf16, tag="x", name=f"xt{it}")
            nc.sync.dma_start(out=x_tile, in_=xp_r[it])
            x_tiles.append(x_tile)
        wk_sbs = {}

        def load_wk(k):
            t = wkp.tile([128, ND, NKQ], bf16, tag=f"wk{k % 2}", name=f"wk{k}")
            nc.sync.dma_start(out=t,
                              in_=wk[k].rearrange("(a p) n -> p a n", p=128))
            wk_sbs[k] = t

        load_wk(0)
        load_wk(1)

        kproj_ps = ctx.enter_context(
            tc.tile_pool(name="kproj_ps", bufs=2, space="PSUM"))
        vals = [valsp.tile([128, NV], bf16, tag=f"v{it}", name=f"vals{it}")
                for it in range(pt)]
        ks_tiles = {0: keysp.tile([128, 2, pt, NKQ], bf16, tag="ks",
                                  name="ks0")}

        # ---- head: per tile, transpose raw x then immediately run the V and
        # op-0 K projections (only their rank-1 correction + scaled copy wait
        # on the LN stats, so PE work rate matches the ACT LN rate)
        with tc.tile_pool(name="tp_ps", bufs=3, space="PSUM") as tp_ps, \
             tc.tile_pool(name="scr_ps", bufs=1, space="PSUM") as scr_ps, \
             tc.tile_pool(name="vproj_ps", bufs=2, space="PSUM") as vproj_ps:
            scr = scr_ps.tile([1, 1], f32)
            nc.tensor.matmul(scr, ident_f[:, 0:1], ident_f[:, 0:1],
                             start=True, stop=True)
            for it in range(pt):
                x_tile = x_tiles[it]
                tsl = slice(it * 128, (it + 1) * 128)
                _emit_ln_stats(nc, mybir, lnp, eps_t, x_tile, it,
                               negmu_row, rstds, 1.0 / D)
                # dummy matmul absorbs semaphore waits so each transpose
                # (1-wait-limited LDWEIGHTS struct) needs at most one wait
                nc.tensor.matmul(scr, ident_f[:, 0:1], ident_f[:, 0:1],
                                 start=True, stop=True)
                for d in range(ND):
                    tp = tp_ps.tile([128, 128], bf16)
                    sl = x_tile[:, d // 4, (d % 4) * 128:(d % 4) * 128 + 128]
                    nc.tensor.transpose(tp, sl, ident)
                    nc.vector.tensor_copy(out=xT[d][:, it * 128:(it + 1) * 128],
                                          in_=tp)
                vp = vproj_ps.tile([128, NV], f32)
                for d in range(ND):
                    mm(vp, xT[d][:, tsl], wv_sb[:, d, :],
                       start=(d == 0), stop=False)
                mm(vp, negmu_row[0:1, tsl], csv_sb, start=False, stop=True)
                nc.scalar.activation(out=vals[it], in_=vp,
                                     func=mybir.ActivationFunctionType.Copy,
                                     bias=0.0, scale=rstds[it][:, 0:1])
                kp = kproj_ps.tile([128, NKQ], f32)
                for d in range(ND):
                    mm(kp, xT[d][:, tsl], wk_sbs[0][:, d, :],
                       start=(d == 0), stop=False)
                mm(kp, negmu_row[0:1, tsl], csk_sb[:, 0, :],
                   start=False, stop=True)
                nc.scalar.activation(out=ks_tiles[0][:, 0, it, :], in_=kp,
                                     func=mybir.ActivationFunctionType.Copy,
                                     bias=0.0, scale=rstds[it][:, 0:1])

        # ---- per-op: shift, per-head G/M/C_v^T with the NEXT op's K-proj
        # tiles interleaved (fills the pack-copy stalls between heads) ----
        with tc.tile_pool(name="gm_ps", bufs=2, space="PSUM") as gm_ps:

            def kproj_tile(kk, it):
                tsl = slice(it * 128, (it + 1) * 128)
                kp = kproj_ps.tile([128, NKQ], f32)
                for d in range(ND):
                    mm(kp, xT[d][:, tsl], wk_sbs[kk][:, d, :],
                       start=(d == 0), stop=False)
                mm(kp, negmu_row[0:1, tsl], csk_sb[:, kk, :],
                   start=False, stop=True)
                nc.scalar.activation(
                    out=ks_tiles[kk][:, 0, it, :], in_=kp,
                    func=mybir.ActivationFunctionType.Copy,
                    bias=0.0, scale=rstds[it][:, 0:1])

            for k in range(K_OPS):
                ks = ks_tiles[k]
                # shifted keys in 3 DMAs (row l holds key l+1; zero padding
                # beyond the prefix makes the tail terms vanish)
                nc.sync.dma_start(out=ks[0:127, 1, :, :], in_=ks[1:128, 0, :, :])
                if pt > 1:
                    nc.sync.dma_start(out=ks[127:128, 1, 0:pt - 1, :],
                                      in_=ks[0:1, 0, 1:pt, :])
                nc.sync.dma_start(out=ks[127:128, 1, pt - 1, :], in_=zrow)

                if k + 1 < K_OPS:
                    ks_tiles[k + 1] = keysp.tile([128, 2, pt, NKQ], bf16,
                                                 tag="ks", name=f"ks{k + 1}")
                if k + 2 < K_OPS:
                    load_wk(k + 2)

                g_pack = gmsb.tile([R, HPC, R], f32, tag="gp", name=f"gpk{k}")
                m_pack = gmsb.tile([R, HPC, R], f32, tag="mp", name=f"mpk{k}")
                c_pack = gmsb.tile([R, HPC, HD], f32, tag="cp", name=f"cpk{k}")
                nsteps = max(HPC, pt if k + 1 < K_OPS else 0)
                for h in range(nsteps):
                    if k + 1 < K_OPS and h < pt:
                        kproj_tile(k + 1, h)
                    if h >= HPC:
                        continue
                    hs = slice(h * R, (h + 1) * R)
                    vs = slice(h * HD, (h + 1) * HD)
                    gps = gm_ps.tile([R, R], f32, tag="g", name=f"g{k}_{h}")
                    mps = gm_ps.tile([R, R], f32, tag="m", name=f"m{k}_{h}")
                    cps = gm_ps.tile([R, HD], f32, tag="c", name=f"c{k}_{h}")
                    for it in range(pt):
                        lhs = ks[:, 0, it, hs]
                        mm(gps, lhs, ks[:, 0, it, hs],
                           start=(it == 0), stop=(it == pt - 1))
                        mm(mps, ks[:, 1, it, hs], ks[:, 0, it, hs],
                           start=(it == 0), stop=(it == pt - 1))
                        mm(cps, lhs, vals[it][:, vs],
                           start=(it == 0), stop=(it == pt - 1))
                    nc.vector.tensor_copy(out=g_pack[:, h, :], in_=gps)
                    nc.vector.tensor_copy(out=m_pack[:, h, :], in_=mps)
                    nc.vector.tensor_copy(out=c_pack[:, h, :], in_=cps)
                    if h == HPC // 2 - 1:
                        nc.sync.dma_start(out=g_out[k, :, 0:HPC // 2],
                                          in_=g_pack[:, 0:HPC // 2, :])
                        nc.sync.dma_start(out=m_out[k, :, 0:HPC // 2],
                                          in_=m_pack[:, 0:HPC // 2, :])
                        nc.sync.dma_start(out=cv_out[k, :, 0:HPC // 2],
                                          in_=c_pack[:, 0:HPC // 2, :])
                nc.sync.dma_start(out=g_out[k, :, HPC // 2:],
                                  in_=g_pack[:, HPC // 2:, :])
                nc.sync.dma_start(out=m_out[k, :, HPC // 2:],
                                  in_=m_pack[:, HPC // 2:, :])
                nc.sync.dma_start(out=cv_out[k, :, HPC // 2:],
                                  in_=c_pack[:, HPC // 2:, :])
    return _split_multi_waits(nc)


def _build_phase2():
    import concourse.bass as bass
    import concourse.tile as tile
    from concourse import mybir
    from concourse.masks import make_identity
    from contextlib import ExitStack

    f32 = mybir.dt.float32
    bf16 = mybir.dt.bfloat16

    nc = bass.Bass()
    xh = nc.dram_tensor("xh", [TH, D], bf16, kind="ExternalInput")
    weff = nc.dram_tensor("weff", [D, H * HD], bf16, kind="ExternalInput")
    wo = nc.dram_tensor("wo", [H * HD, D], bf16, kind="ExternalInput")
    cwo = nc.dram_tensor("cwo", [1, D], bf16, kind="ExternalInput")
    y_out = nc.dram_tensor("y_out", [TH, D], bf16, kind="ExternalOutput")

    mm = nc.tensor.matmul
    NHD = (H * HD) // 128  # 8 head-dim chunks

    with tile.TileContext(nc) as tc, ExitStack() as ctx:
        const = ctx.enter_context(tc.tile_pool(name="const", bufs=1))
        wp = ctx.enter_context(tc.tile_pool(name="wp", bufs=1))
        xtp = ctx.enter_context(tc.tile_pool(name="xtp", bufs=1))
        xin = ctx.enter_context(tc.tile_pool(name="xin", bufs=1))
        lnp = ctx.enter_context(tc.tile_pool(name="lnp", bufs=3))
        rsp = ctx.enter_context(tc.tile_pool(name="rsp", bufs=1))
        ysb = ctx.enter_context(tc.tile_pool(name="ysb", bufs=3))

        ident_f = const.tile([128, 128], f32)
        make_identity(nc, ident_f)
        ident = const.tile([128, 128], bf16)
        nc.vector.tensor_copy(out=ident, in_=ident_f)
        eps_t = const.tile([128, 1], f32)
        nc.vector.memset(eps_t, LN_EPS)
        negmu_row = const.tile([1, TH], bf16)
        rstds = [rsp.tile([128, 1], f32, tag=f"r{it}", name=f"rstd{it}")
                 for it in range(NTH)]

        xT = [xtp.tile([128, TH], bf16, tag=f"xT{d}", name=f"xT{d}")
              for d in range(ND)]
        accT = [xtp.tile([128, TH], bf16, tag=f"aT{j}", name=f"accT{j}")
                for j in range(NHD)]
        weff_sb = [wp.tile([128, H * HD], bf16, tag=f"we{i}", name=f"we{i}")
                   for i in range(ND)]
        wo_sb = wp.tile([128, NHD, D], bf16)
        cwo_sb = const.tile([1, D], bf16)

        xh_r = xh.rearrange("(t p) (a b) -> t p a b", p=128, a=2)

        # DMA order: x tiles gate the transposes from t=0; weff chunks gate
        # accT; wo (by column halves) gates the y stage
        x_tiles = []

        def load_x(it):
            x_tile = xin.tile([128, 2, 512], bf16, tag=f"x{it}", name=f"xt{it}")
            if it == 0:
                nc.sync.dma_start(out=x_tile[:, 0, :], in_=xh_r[it][:, 0, :])
                nc.sync.dma_start(out=x_tile[:, 1, :], in_=xh_r[it][:, 1, :])
            else:
                nc.sync.dma_start(out=x_tile, in_=xh_r[it])
            x_tiles.append(x_tile)

        for it in range(4):
            load_x(it)
        nc.sync.dma_start(out=cwo_sb, in_=cwo[:, :])
        for i in range(ND):
            nc.sync.dma_start(out=weff_sb[i],
                              in_=weff[i * 128:(i + 1) * 128, :])
        for it in range(4, NTH):
            load_x(it)
        wo_r = wo.rearrange("(a p) n -> p a n", p=128)
        nc.sync.dma_start(out=wo_sb[:, :, 0:512], in_=wo_r[:, :, 0:512])
        nc.sync.dma_start(out=wo_sb[:, :, 512:1024], in_=wo_r[:, :, 512:1024])

        # transposes(tchunk) then accT(tchunk): the i-outer accumulation
        # streams with the weff chunk arrivals (4 PSUM banks per half-pass),
        # so accT(t0) starts ~4us in; y after both, once wo lands
        with tc.tile_pool(name="tp_ps", bufs=3, space="PSUM") as tp_ps, \
             tc.tile_pool(name="scr_ps", bufs=1, space="PSUM") as scr_ps, \
             tc.tile_pool(name="acc_ps", bufs=1, space="PSUM") as acc_ps:
            scr = scr_ps.tile([1, 1], f32)
            nc.tensor.matmul(scr, ident_f[:, 0:1], ident_f[:, 0:1],
                             start=True, stop=True)
            for it in range(NTH):
                _emit_ln_stats(nc, mybir, lnp, eps_t, x_tiles[it], it,
                               negmu_row, rstds, 1.0 / D)

            def transpose_tiles(lo, hi):
                for it in range(lo, hi):
                    nc.tensor.matmul(scr, ident_f[:, 0:1], ident_f[:, 0:1],
                                     start=True, stop=True)
                    for d in range(ND):
                        tp = tp_ps.tile([128, 128], bf16)
                        sl = x_tiles[it][:, d // 4,
                                         (d % 4) * 128:(d % 4) * 128 + 128]
                        nc.tensor.transpose(tp, sl, ident)
                        nc.vector.tensor_copy(
                            out=xT[d][:, it * 128:(it + 1) * 128], in_=tp)

            def acc_half(tch, jh):
                tsl = slice(tch * 512, (tch + 1) * 512)
                pss = [acc_ps.tile([128, 512], f32, tag=f"a{jj}",
                                   name=f"acc{tch}_{jh}_{jj}")
                       for jj in range(4)]
                for i in range(ND):
                    for jj in range(4):
                        j = jh * 4 + jj
                        mm(pss[jj], weff_sb[i][:, j * 128:(j + 1) * 128],
                           xT[i][:, tsl],
                           start=(i == 0), stop=(i == ND - 1))
                for jj in range(4):
                    nc.vector.tensor_copy(
                        out=accT[jh * 4 + jj][:, tsl], in_=pss[jj])

            # transposes(4-7) slot between the accT(t0) halves so the PE
            # stream never waits on a stage boundary
            transpose_tiles(0, 4)
            acc_half(0, 0)
            transpose_tiles(4, NTH)
            acc_half(0, 1)
            acc_half(1, 0)
            acc_half(1, 1)

        # y[tile] = rstd * sum_j acc^T[j, tile]^T @ W_O[j-chunk, :]
        with tc.tile_pool(name="y_ps", bufs=3, space="PSUM") as y_ps:
            y_r = y_out.rearrange("p (a b) -> p a b", a=2)
            for it in range(NTH):
                ysl = slice(it * 128, (it + 1) * 128)
                y_sb = ysb.tile([128, 2, 512], bf16)
                for ch in range(D // 512):
                    csl = slice(ch * 512, (ch + 1) * 512)
                    ps = y_ps.tile([128, 512], f32)
                    for j in range(NHD):
                        mm(ps, accT[j][:, ysl], wo_sb[:, j, csl],
                           start=(j == 0), stop=False)
                    mm(ps, negmu_row[0:1, ysl], cwo_sb[0:1, csl],
                       start=False, stop=True)
                    nc.scalar.activation(out=y_sb[:, ch, :], in_=ps,
                                         func=mybir.ActivationFunctionType.Copy,
                                         bias=0.0, scale=rstds[it][:, 0:1])
                    nc.sync.dma_start(out=y_r[ysl, ch], in_=y_sb[:, ch, :])
    return _split_multi_waits(nc)


def _numpy_fallback(hidden_states, W_K_ops, W_Q_ops, W_V, W_O, ln_gamma, ln_beta,
                    gate_alphas, gate_alpha, log_ridges, log_gammas, pl):
    x = np.asarray(hidden_states, np.float64)
    mu = x.mean(-1, keepdims=True)
    var = x.var(-1, keepdims=True)
    normed = (x - mu) / np.sqrt(var + LN_EPS) * ln_gamma + ln_beta
    values = (normed @ W_V).reshape(B, T, H, HD).transpose(0, 2, 1, 3)
    acc = np.zeros((B, H, T, HD))
    eye = np.eye(R)
    for k in range(K_OPS):
        ridge = math.exp(float(log_ridges[k]))
        gamma = math.exp(float(log_gammas[k]))
        gate = 1.0 / (1.0 + math.exp(-float(gate_alphas[k])))
        keys = (normed @ W_K_ops[k]).reshape(B, T, H, R).transpose(0, 2, 1, 3)
        qs = (normed @ W_Q_ops[k]).reshape(B, T, H, R).transpose(0, 2, 1, 3)
        pk = keys[:, :, :pl, :]
        G = np.einsum('bhlr,bhls->bhrs', pk, pk) + ridge * eye
        M = np.einsum('bhlr,bhls->bhrs', pk[:, :, 1:, :], pk[:, :, :-1, :])
        L = np.linalg.cholesky(G)
        Linv = np.linalg.inv(L)
        A = Linv @ M @ np.swapaxes(Linv, -1, -2)
        sig = np.linalg.svd(A, compute_uv=False)[..., 0]
        sig = np.maximum(sig, 1e-8)
        scale = min(gamma, 1.0) / np.maximum(sig, 1.0)
        A = A * scale[..., None, None]
        pv = values[:, :, :pl, :]
        Cv = np.einsum('bhld,bhlr->bhdr', pv, pk)
        Ginv = np.swapaxes(Linv, -1, -2) @ Linv
        Bv = Cv @ Ginv
        E = Bv @ L @ A @ A @ Linv
        out_k = np.einsum('bhdr,bhtr->bhtd', E, qs)
        acc = acc + gate * out_k
    out = acc.transpose(0, 2, 1, 3).reshape(B, T, H * HD) @ W_O
    sg = 1.0 / (1.0 + math.exp(-float(np.asarray(gate_alpha).ravel()[0])))
    return (sg * out).astype(np.float32)


def _extract_gmcv(res):
    """host-side: unpack the per-head [R, HPC, *] device outputs."""
    G = res["g_out"].transpose(0, 2, 1, 3).astype(np.float64)    # [K,HPC,R,R]
    M = res["m_out"].transpose(0, 2, 1, 3).astype(np.float64)
    Cvt = res["cv_out"].transpose(0, 2, 1, 3).astype(np.float64)  # [K,HPC,R,HD]
    return G, M, np.swapaxes(Cvt, -1, -2)


def kernel(hidden_states, W_K_ops, W_Q_ops, W_V, W_O, ln_gamma, ln_beta,
           gate_alphas, gate_alpha, log_ridges, log_gammas, prefix_len):
    from concourse.bass_utils import run_bass_kernel_spmd

    hidden_states = np.ascontiguousarray(np.asarray(hidden_states, np.float32))
    W_K_ops = np.asarray(W_K_ops, np.float32)
    W_Q_ops = np.asarray(W_Q_ops, np.float32)
    W_V = np.asarray(W_V, np.float32)
    W_O = np.ascontiguousarray(np.asarray(W_O, np.float32))
    ln_gamma = np.asarray(ln_gamma, np.float32)
    ln_beta = np.asarray(ln_beta, np.float32)
    gate_alphas = np.asarray(gate_alphas, np.float32)
    log_ridges = np.asarray(log_ridges, np.float32)
    log_gammas = np.asarray(log_gammas, np.float32)
    pl = max(1, min(int(prefix_len), T - 1))
    pt = (pl + 127) // 128

    if np.any(ln_beta != 0) or pl < 2:
        return _numpy_fallback(hidden_states, W_K_ops, W_Q_ops, W_V, W_O,
                               ln_gamma, ln_beta, gate_alphas, gate_alpha,
                               log_ridges, log_gammas, pl)

    # fold LN gamma into the projection weights; bf16 device copies
    wk_f = W_K_ops * ln_gamma[None, :, None]
    wq_f = W_Q_ops * ln_gamma[None, :, None]
    wv_f = W_V * ln_gamma[:, None]

    if pl == pt * 128:
        xpad = hidden_states[:, :pl]
    else:
        xpad = np.zeros((B, pt * 128, D), np.float32)
        xpad[:, :pl] = hidden_states[:, :pl]
    xpad_b = _bf16(xpad)

    in1 = []
    for c in range(NCORES):
        b, h0 = c // 2, (c % 2) * HPC
        wk_c = _bf16(wk_f[:, :, h0 * R:(h0 + HPC) * R])
        wv_c = _bf16(wv_f[:, h0 * HD:(h0 + HPC) * HD])
        in1.append({
            "xp": xpad_b[b],
            "wk": np.ascontiguousarray(wk_c),
            "wv": np.ascontiguousarray(wv_c),
            "csk": np.ascontiguousarray(
                wk_c.astype(np.float32).sum(1)[:, None, :]).astype(wk_c.dtype),
            "csv": np.ascontiguousarray(
                wv_c.astype(np.float32).sum(0)[None, :]).astype(wv_c.dtype),
        })

    key1 = ("p1", pt)
    if key1 not in _cache:
        _cache[key1] = _build_phase1(pt)
    r1 = run_bass_kernel_spmd(_cache[key1], in1, core_ids=list(range(NCORES)))
    LAST_PERF["p1"] = r1

    # ---- host linear algebra on 48x48 blocks -> W_eff per batch ----
    ridge = np.exp(log_ridges.astype(np.float64))
    gamma_k = np.exp(log_gammas.astype(np.float64))
    gates = 1.0 / (1.0 + np.exp(-gate_alphas.astype(np.float64)))
    sg = 1.0 / (1.0 + math.exp(-float(np.asarray(gate_alpha).ravel()[0])))
    eye = np.eye(R)

    E_full = np.empty((B, K_OPS, H, HD, R))
    for c in range(NCORES):
        b, h0 = c // 2, (c % 2) * HPC
        G, M, Cv = _extract_gmcv(r1.results[c])
        G = G + ridge[:, None, None, None] * eye
        L = np.linalg.cholesky(G)
        Linv = np.linalg.inv(L)
        A = Linv @ M @ np.swapaxes(Linv, -1, -2)
        sig = np.linalg.svd(A, compute_uv=False)[..., 0]
        sig = np.maximum(sig, 1e-8)
        scale = np.minimum(gamma_k, 1.0)[:, None] / np.maximum(sig, 1.0)
        A = A * scale[..., None, None]
        Ginv = np.swapaxes(Linv, -1, -2) @ Linv
        Bv = Cv @ Ginv
        E = Bv @ L @ A @ A @ Linv          # [K, HPC, HD, R]
        E = E * (sg * gates)[:, None, None, None]
        E_full[b, :, h0:h0 + HPC] = E

    # W_eff[b] = sum_k Wq_f[k][:, h-block] @ E[b, k, h]^T   -> [D, H*HD]
    wq_h = wq_f.reshape(K_OPS, D, H, R).transpose(0, 2, 1, 3)  # [K, H, D, R]
    weffs = []
    for b in range(B):
        w = np.zeros((H, D, HD), np.float64)
        for k in range(K_OPS):
            w += wq_h[k].astype(np.float64) @ E_full[b, k].transpose(0, 2, 1)
        weffs.append(np.ascontiguousarray(
            w.transpose(1, 0, 2).reshape(D, H * HD)))

    xh_b = _bf16(hidden_states)
    wo_b = _bf16(W_O)
    in2 = []
    for c in range(NCORES):
        b, th = c // 2, c % 2
        weff_b = _bf16(weffs[b])
        in2.append({
            "xh": np.ascontiguousarray(xh_b[b, th * TH:(th + 1) * TH]),
            "weff": weff_b,
            "wo": wo_b,
            "cwo": np.ascontiguousarray(
                (weffs[b].sum(0) @ W_O.astype(np.float64))[None, :]
                .astype(np.float32)).astype(weff_b.dtype),
        })

    if "p2" not in _cache:
        _cache["p2"] = _build_phase2()
    r2 = run_bass_kernel_spmd(_cache["p2"], in2, core_ids=list(range(NCORES)))
    LAST_PERF["p2"] = r2

    y = np.empty((B, T, D), np.float32)
    for c in range(NCORES):
        b, th = c // 2, c % 2
        y[b, th * TH:(th + 1) * TH] = r2.results[c]["y_out"].astype(np.float32)
    return y
